# revision 1
# baseline (speedup 1.0000x reference)
"""MoE pointwise conv2d kernel for Trainium2 (8 NeuronCores, SPMD data-parallel).

Problem: out[b,o,h,w] = sum_i (sum_e routing[b,e] * weight[e,o,i]) * x[b,i,h,w]
Shapes:  x [64,384,28,28] f32, routing [64,8] f32, weight [8,384,384] f32.

Strategy (per core, 8 samples each):
  - Routing-combine (agg^T[b][i,o] = sum_e r[b,e] * w[e,o,i]) split across
    VectorE and GpSimdE via fused scalar_tensor_tensor MACs, written directly
    in matmul-lhsT layout (partition = i, free = (ki, o)).
  - Per-sample GEMM out[b] = agg_b @ x_b on TensorE, accumulating over 3
    k-tiles in PSUM (fp32).
  - ScalarE evacuates PSUM -> SBUF; HWDGE DMAs stream x in / out back.
  - Default fp16 wire format (x/weights/out on HBM + agg math) halves DMA
    volume and doubles DVE throughput; end-to-end rel err ~7e-4.
    KERNEL_F32=1 selects the fp32(+float32r matmul) build, rel err ~1.6e-4.
"""
import os
import sys

sys.path.insert(0, "/opt/trn_rl_repo")

import numpy as np
from contextlib import ExitStack

B, C_IN, C_OUT, E, H, W = 64, 384, 384, 8, 28, 28
HW = H * W            # 784
N_CORES = 8
BPC = B // N_CORES    # 8 samples per core
KI = C_IN // 128      # 3 k-tiles
MO = C_OUT // 128     # 3 output-partition tiles
NSPLIT = 2            # 784 -> 2 x 392 (<= 512 psum bank limit)
NCH = HW // NSPLIT    # 392
WCOL = KI * C_OUT     # 1152

USE_F16 = os.environ.get("KERNEL_F32", "0") != "1"

_cache = {}


def _build(use_f16=USE_F16, spl=WCOL, reps=1, serialize_reps=False, pair=True, agg_bufs=2, micro=True, quad=False, dense_rw=False, slack=True, slack2=False):
    import concourse.tile as tile
    import concourse.mybir as mybir
    from concourse import bacc
    from concourse.tile import add_dep_helper

    f32 = mybir.dt.float32
    f32r = mybir.dt.float32r
    f16 = mybir.dt.float16
    mult = mybir.AluOpType.mult
    add = mybir.AluOpType.add

    dio = f16 if use_f16 else f32        # wire dtype for wt/x/out
    dmm = f16 if use_f16 else f32r       # matmul operand dtype

    nc = bacc.Bacc("TRN2", target_bir_lowering=False, debug=False)
    x_d = nc.dram_tensor("x", [BPC, C_IN, HW], dio, kind="ExternalInput")
    rw_d = nc.dram_tensor("rw", [128 if dense_rw else 1, BPC * E], f32,
                          kind="ExternalInput")
    wt_d = nc.dram_tensor("wt", [E, 128, WCOL], dio, kind="ExternalInput")
    out_d = nc.dram_tensor("out", [reps * BPC, C_OUT, HW], dio,
                           kind="ExternalOutput")

    with tile.TileContext(nc) as tc:
        with ExitStack() as ctx:
            wt_pool = ctx.enter_context(tc.tile_pool(name="wt", bufs=E))
            rw_pool = ctx.enter_context(tc.tile_pool(name="rw", bufs=2))
            agg_pool = ctx.enter_context(tc.tile_pool(name="agg", bufs=max(agg_bufs, 4 if quad else (3 if slack2 else 2))))
            x_pool = ctx.enter_context(tc.tile_pool(name="xp", bufs=4 if slack2 else (3 if slack else 2)))
            out_pool = ctx.enter_context(tc.tile_pool(name="op", bufs=10 if slack2 else (8 if slack else 6)))
            ps_pool = ctx.enter_context(tc.tile_pool(name="ps", bufs=8 if slack2 else (6 if slack else 4), space="PSUM"))

            prev_out_dmas, cur_out_dmas = [], []
            pair_tiles = {}

            def _fence(inst):
                if serialize_reps:
                    for d in prev_out_dmas:
                        add_dep_helper(inst.ins, d.ins, reason="serialize reps")
                return inst

            for rep, b in ((r, b) for r in range(reps) for b in range(BPC)):
                if b == 0:
                    prev_out_dmas, cur_out_dmas = cur_out_dmas, []
                    rw_sb = rw_pool.tile([128, BPC * E], f32)
                    _fence(nc.sync.dma_start(
                        rw_sb[:],
                        rw_d[:] if dense_rw
                        else rw_d[:].to_broadcast((128, BPC * E))))
                    wt_sb, wt_dmas = [], []
                    for e in range(E):
                        t = wt_pool.tile([128, WCOL], dio)
                        wt_dmas.append(_fence(nc.sync.dma_start(t[:], wt_d[e])))
                        wt_sb.append(t)
                # ---- routing combine ----
                # DVE does cols [0:spl) with fused scalar_tensor_tensor MACs
                # (2-byte operands keep the 2x_1p DVE mode). GPSIMD cannot run
                # TensorScalarPtr (walrus rejects Pool), and its tensor_tensor
                # 2-op MAC measured ~33us/invocation WORSE on HW (shared-port
                # lock vs DVE packed modes) — keep spl == WCOL (DVE-only).
                # fp16 accumulator keeps every operand 2-byte -> 2x DVE mode
                GSZ = 4 if quad else 2
                if pair and b % GSZ == 0:
                    # emit the MAC chains of samples (b, b+1) interleaved so
                    # DVE hides each chain's op-to-op dependency latency
                    pr = []
                    for bb in range(b, b + GSZ):
                        a_ = agg_pool.tile([128, WCOL], f16 if use_f16 else f32,
                                           tag="aggT")
                        ar_ = agg_pool.tile([128, WCOL], dmm, tag="aggr")
                        pr.append((bb, a_, ar_))
                    for gi in range(GSZ):
                        pair_tiles[b + gi] = pr[gi][1:]
                    for e in range(E):
                        for bb, a_, ar_ in pr:
                            s = rw_sb[:, bb * E + e:bb * E + e + 1]
                            if e == 0:
                                nc.vector.tensor_scalar(
                                    a_[:], wt_sb[0][:], s, None, mult)
                            elif e < E - 1:
                                nc.vector.scalar_tensor_tensor(
                                    a_[:], wt_sb[e][:], s, a_[:], mult, add)
                            elif micro and b == BPC - GSZ:
                                for k3 in range(KI):
                                    cs = slice(k3 * C_OUT, (k3 + 1) * C_OUT)
                                    nc.vector.scalar_tensor_tensor(
                                        ar_[:, cs], wt_sb[e][:, cs], s,
                                        a_[:, cs], mult, add)
                            else:
                                nc.vector.scalar_tensor_tensor(
                                    ar_[:], wt_sb[e][:], s, a_[:], mult, add)
                if pair:
                    aggT, aggT_r = pair_tiles[b]
                    sc = lambda e: rw_sb[:, b * E + e:b * E + e + 1]
                else:
                    aggT = agg_pool.tile([128, WCOL], f16 if use_f16 else f32)
                    aggT_r = agg_pool.tile([128, WCOL], dmm, tag="aggr")
                    sc = lambda e: rw_sb[:, b * E + e:b * E + e + 1]
                if not pair:
                    nc.vector.tensor_scalar(
                        aggT[:, 0:spl], wt_sb[0][:, 0:spl], sc(0), None, mult
                    )
                    for e in range(1, E - 1):
                        nc.vector.scalar_tensor_tensor(
                            aggT[:, 0:spl], wt_sb[e][:, 0:spl], sc(e),
                            aggT[:, 0:spl], mult, add,
                        )
                    nc.vector.scalar_tensor_tensor(
                        aggT_r[:, 0:spl], wt_sb[E - 1][:, 0:spl], sc(E - 1),
                        aggT[:, 0:spl], mult, add,
                    )
                if spl < WCOL:
                    gw = WCOL - spl
                    gtmp = agg_pool.tile([128, gw], f16 if use_f16 else f32,
                                         tag="gtmp")
                    scb = lambda e: sc(e).to_broadcast((128, gw))
                    nc.gpsimd.tensor_tensor(
                        aggT[:, spl:], wt_sb[0][:, spl:], scb(0), mult)
                    for e in range(1, E - 1):
                        nc.gpsimd.tensor_tensor(
                            gtmp[:], wt_sb[e][:, spl:], scb(e), mult)
                        nc.gpsimd.tensor_tensor(
                            aggT[:, spl:], aggT[:, spl:], gtmp[:], add)
                    nc.gpsimd.tensor_tensor(
                        gtmp[:], wt_sb[E - 1][:, spl:], scb(E - 1), mult)
                    nc.gpsimd.tensor_tensor(
                        aggT_r[:, spl:], aggT[:, spl:], gtmp[:], add)

                # ---- load x_b ----
                x_sb = x_pool.tile([128, KI * HW], dmm)
                for ki in range(KI):
                    src = x_d[b, ki * 128:(ki + 1) * 128, :]
                    xi = _fence(nc.sync.dma_start(x_sb[:, ki * HW:(ki + 1) * HW],
                                                  src if use_f16 else src.bitcast(f32r)))
                    if micro and b < 2:
                        for wd in wt_dmas:
                            add_dep_helper(xi.ins, wd.ins,
                                           reason="x after wt (head trim)")

                # ---- per-sample GEMM ----
                for mo in range(MO):
                    for n in range(NSPLIT):
                        ps = ps_pool.tile([128, NCH], f32)
                        for ki in range(KI):
                            lhs = aggT_r[:, ki * C_OUT + mo * 128:
                                         ki * C_OUT + (mo + 1) * 128]
                            rhs = x_sb[:, ki * HW + n * NCH:
                                       ki * HW + (n + 1) * NCH]
                            nc.tensor.matmul(
                                ps[:], lhs, rhs,
                                start=(ki == 0), stop=(ki == KI - 1),
                            )
                        o_sb = out_pool.tile([128, NCH], dio)
                        nc.scalar.copy(o_sb[:], ps[:])
                        cur_out_dmas.append(nc.sync.dma_start(
                            out_d[rep * BPC + b, mo * 128:(mo + 1) * 128,
                                  n * NCH:(n + 1) * NCH],
                            o_sb[:],
                        ))
    nc.compile()
    return nc


def kernel(x: np.ndarray, routing_weights: np.ndarray, weight: np.ndarray,
           _trace: bool = False):
    from concourse.bass_utils import run_bass_kernel_spmd

    x = np.asarray(x, dtype=np.float32)
    routing_weights = np.ascontiguousarray(np.asarray(routing_weights, dtype=np.float32))
    weight = np.asarray(weight, dtype=np.float32)

    if "nc" not in _cache:
        _cache["nc"] = _build()
    nc = _cache["nc"]

    np_io = np.float16 if USE_F16 else np.float32

    # wt[e, p, ki*384 + o] = weight[e, o, ki*128 + p]
    wt = np.ascontiguousarray(
        weight.reshape(E, C_OUT, KI, 128).transpose(0, 3, 2, 1)
        .reshape(E, 128, WCOL).astype(np_io)
    )
    x_r = np.ascontiguousarray(x.reshape(B, C_IN, HW).astype(np_io))

    in_maps = []
    for c in range(N_CORES):
        sl = slice(c * BPC, (c + 1) * BPC)
        in_maps.append({
            "x": x_r[sl],
            "rw": np.ascontiguousarray(routing_weights[sl].reshape(1, BPC * E)),
            "wt": wt,
        })

    res = run_bass_kernel_spmd(nc, in_maps, core_ids=list(range(N_CORES)),
                               trace=_trace)
    out = np.concatenate([res.results[c]["out"] for c in range(N_CORES)], axis=0)
    if _trace:
        _cache["last_result"] = res
    return out.reshape(B, C_OUT, H, W).astype(np.float32)


if __name__ == "__main__":
    rng = np.random.default_rng(0)
    x = rng.standard_normal((B, C_IN, H, W), dtype=np.float32)
    rw = rng.random((B, E), dtype=np.float32)
    w = rng.standard_normal((E, C_OUT, C_IN), dtype=np.float32)
    got = kernel(x, rw, w)
    agg = np.einsum('be,eoi->boi', rw, w)
    want = np.einsum('boi,bihw->bohw', agg, x.reshape(B, C_IN, H, W))
    err = np.abs(got - want).max() / np.abs(want).max()
    print("rel err:", err)



# revision 30
# speedup vs baseline: 1.0130x; 1.0130x over previous
"""MoE pointwise conv2d kernel for Trainium2 (8 NeuronCores, SPMD data-parallel).

Problem: out[b,o,h,w] = sum_i (sum_e routing[b,e] * weight[e,o,i]) * x[b,i,h,w]
Shapes:  x [64,384,28,28] f32, routing [64,8] f32, weight [8,384,384] f32.

Strategy (per core, 8 samples each):
  - Routing-combine (agg^T[b][i,o] = sum_e r[b,e] * w[e,o,i]) split across
    VectorE and GpSimdE via fused scalar_tensor_tensor MACs, written directly
    in matmul-lhsT layout (partition = i, free = (ki, o)).
  - Per-sample GEMM out[b] = agg_b @ x_b on TensorE, accumulating over 3
    k-tiles in PSUM (fp32).
  - ScalarE evacuates PSUM -> SBUF; HWDGE DMAs stream x in / out back.
  - Default fp16 wire format (x/weights/out on HBM + agg math) halves DMA
    volume and doubles DVE throughput; end-to-end rel err ~7e-4.
    KERNEL_F32=1 selects the fp32(+float32r matmul) build, rel err ~1.6e-4.
"""
import os
import sys

sys.path.insert(0, "/opt/trn_rl_repo")

import numpy as np
from contextlib import ExitStack

B, C_IN, C_OUT, E, H, W = 64, 384, 384, 8, 28, 28
HW = H * W            # 784
N_CORES = 8
BPC = B // N_CORES    # 8 samples per core
KI = C_IN // 128      # 3 k-tiles
MO = C_OUT // 128     # 3 output-partition tiles
NSPLIT = 2            # 784 -> 2 x 392 (<= 512 psum bank limit)
NCH = HW // NSPLIT    # 392
WCOL = KI * C_OUT     # 1152

USE_F16 = os.environ.get("KERNEL_F32", "0") != "1"

_cache = {}


def _build(use_f16=USE_F16, spl=WCOL, reps=1, serialize_reps=False, pair=True, agg_bufs=2, micro=True, quad=False, dense_rw=False, slack=True, slack2=False, probe="", combine="twoop", act_mults=0, final_pool=False, evac_dve=0, out_merge=None, act_e0=True, pool_final=False, act_last=8, act_third=0):
    if out_merge is None:
        out_merge = combine == "split"
    if combine == "split":
        agg_bufs = max(agg_bufs, 4)
    import concourse.tile as tile
    import concourse.mybir as mybir
    from concourse import bacc
    from concourse.tile import add_dep_helper

    f32 = mybir.dt.float32
    f32r = mybir.dt.float32r
    f16 = mybir.dt.float16
    mult = mybir.AluOpType.mult
    add = mybir.AluOpType.add

    dio = f16 if use_f16 else f32        # wire dtype for wt/x/out
    dmm = f16 if use_f16 else f32r       # matmul operand dtype

    nc = bacc.Bacc("TRN2", target_bir_lowering=False, debug=False)
    x_d = nc.dram_tensor("x", [BPC, C_IN, HW], dio, kind="ExternalInput")
    rw_d = nc.dram_tensor("rw", [128 if dense_rw else 1, BPC * E], f32,
                          kind="ExternalInput")
    wt_d = nc.dram_tensor("wt", [E, 128, WCOL], dio, kind="ExternalInput")
    # out shape independent of reps: bench reps all write the same region
    # (serialized by fences), so the PJRT zero-output staging cost stays
    # constant across reps variants and cancels in the slope.
    out_d = nc.dram_tensor("out", [BPC, C_OUT, HW], dio,
                           kind="ExternalOutput")

    with tile.TileContext(nc) as tc:
        with ExitStack() as ctx:
            wt_pool = ctx.enter_context(tc.tile_pool(name="wt", bufs=E))
            rw_pool = ctx.enter_context(tc.tile_pool(name="rw", bufs=2))
            agg_pool = ctx.enter_context(tc.tile_pool(name="agg", bufs=max(agg_bufs, 4 if quad else (3 if slack2 else 2))))
            x_pool = ctx.enter_context(tc.tile_pool(name="xp", bufs=4 if slack2 else (3 if slack else 2)))
            out_pool = ctx.enter_context(tc.tile_pool(name="op", bufs=10 if slack2 else (8 if slack else 6)))
            ps_pool = ctx.enter_context(tc.tile_pool(name="ps", bufs=8 if slack2 else (6 if slack else 4), space="PSUM"))

            prev_out_dmas, cur_out_dmas = [], []
            pair_tiles = {}

            def _fence(inst):
                if serialize_reps:
                    for d in prev_out_dmas:
                        add_dep_helper(inst.ins, d.ins, reason="serialize reps")
                return inst

            for rep, b in ((r, b) for r in range(reps) for b in range(BPC)):
                if b == 0:
                    prev_out_dmas, cur_out_dmas = cur_out_dmas, []
                    rw_sb = rw_pool.tile([128, BPC * E], f32)
                    _fence(nc.sync.dma_start(
                        rw_sb[:],
                        rw_d[:] if dense_rw
                        else rw_d[:].to_broadcast((128, BPC * E))))
                    wt_sb, wt_dmas = [], []
                    for e in range(E):
                        t = wt_pool.tile([128, WCOL], dio)
                        wt_dmas.append(_fence(nc.sync.dma_start(t[:], wt_d[e])))
                        wt_sb.append(t)
                    if act_e0 and combine == "twoop":
                        # ACT precomputes every sample's e0 product at rep
                        # top (ACT is idle until the first PSUM evac), so
                        # DVE's chain starts at e1 with no stall.
                        actp = {}
                        for bb in range(BPC):
                            p0 = agg_pool.tile([128, WCOL],
                                               f16 if use_f16 else f32,
                                               tag="actp", bufs=BPC + 2)
                            nc.scalar.mul(p0[:], wt_sb[0][:],
                                          rw_sb[:, bb * E:bb * E + 1])
                            actp[bb] = p0
                    if act_third and combine == "twoop":
                        actr = {}
                        for bb in range(act_third):
                            p1 = agg_pool.tile([128, WCOL],
                                               f16 if use_f16 else f32,
                                               tag="actr", bufs=act_third + 2)
                            nc.scalar.mul(p1[:], wt_sb[1][:],
                                          rw_sb[:, bb * E + 1:bb * E + 2])
                            actr[bb] = p1
                    if act_last and combine == "twoop":
                        actq = {}
                        for bb in range(act_last):
                            p7 = agg_pool.tile([128, WCOL],
                                               f16 if use_f16 else f32,
                                               tag="actq", bufs=act_last + 2)
                            nc.scalar.mul(p7[:], wt_sb[E - 1][:],
                                          rw_sb[:, bb * E + E - 1:
                                                bb * E + E])
                            actq[bb] = p7
                # ---- routing combine ----
                # DVE does cols [0:spl) with fused scalar_tensor_tensor MACs
                # (2-byte operands keep the 2x_1p DVE mode). GPSIMD cannot run
                # TensorScalarPtr (walrus rejects Pool), and its tensor_tensor
                # 2-op MAC measured ~33us/invocation WORSE on HW (shared-port
                # lock vs DVE packed modes) — keep spl == WCOL (DVE-only).
                # fp16 accumulator keeps every operand 2-byte -> 2x DVE mode
                GSZ = 4 if quad else 2
                emit_combine = (b == 0) if "dve0" in probe else (b % GSZ == 0)
                if pair and emit_combine:
                    # emit the MAC chains of samples (b, b+1) interleaved so
                    # DVE hides each chain's op-to-op dependency latency
                    pr = []
                    for bb in range(b, b + GSZ):
                        if combine == "twoop2":
                            pr.append((bb, None, None, None))
                            continue
                        a_ = agg_pool.tile([128, WCOL], f16 if use_f16 else f32,
                                           tag="aggT")
                        ar_ = agg_pool.tile([128, WCOL], dmm, tag="aggr")
                        if combine in ("twoop", "split"):
                            t_ = agg_pool.tile([128, WCOL],
                                               f16 if use_f16 else f32,
                                               tag="tmp", bufs=4)
                        else:
                            t_ = None
                        pr.append((bb, a_, ar_, t_))
                    if combine != "twoop2":
                        for gi in range(GSZ):
                            pair_tiles[b + gi] = pr[gi][1:3]
                    if combine == "split":
                        # 3-engine combine: products via tensor_scalar on
                        # DVE (4x mode) + a few on ACT (activation scale);
                        # add-tree split between a DVE chain (e0..e3) and a
                        # Pool chain (e4..e7); final merge DVE or Pool.
                        fdt = f16 if use_f16 else f32
                        ext = []
                        for bb, a_, ar_, t_ in pr:
                            q_ = agg_pool.tile([128, WCOL], fdt, tag="qc")
                            u_ = agg_pool.tile([128, WCOL], fdt, tag="uc")
                            v_ = agg_pool.tile([128, WCOL], fdt, tag="vc")
                            ext.append((bb, a_, ar_, t_, q_, u_, v_))

                        def smul(dst, e, bb, on_act):
                            s = rw_sb[:, bb * E + e:bb * E + e + 1]
                            if on_act:
                                nc.scalar.mul(dst[:], wt_sb[e][:], s)
                            else:
                                nc.vector.tensor_scalar(
                                    dst[:], wt_sb[e][:], s, None, mult)

                        for bb, a_, ar_, t_, q_, u_, v_ in ext:
                            smul(q_, 4, bb, act_mults >= 1)
                        for bb, a_, ar_, t_, q_, u_, v_ in ext:
                            smul(u_, 5, bb, act_mults >= 2)
                        for bb, a_, ar_, t_, q_, u_, v_ in ext:
                            smul(a_, 0, bb, False)
                        for bb, a_, ar_, t_, q_, u_, v_ in ext:
                            smul(t_, 1, bb, False)
                        for bb, a_, ar_, t_, q_, u_, v_ in ext:
                            nc.vector.tensor_tensor(a_[:], a_[:], t_[:], add)
                        for bb, a_, ar_, t_, q_, u_, v_ in ext:
                            nc.gpsimd.tensor_tensor(q_[:], q_[:], u_[:], add)
                        for bb, a_, ar_, t_, q_, u_, v_ in ext:
                            smul(v_, 6, bb, act_mults >= 3)
                        for bb, a_, ar_, t_, q_, u_, v_ in ext:
                            smul(t_, 2, bb, False)
                        for bb, a_, ar_, t_, q_, u_, v_ in ext:
                            nc.vector.tensor_tensor(a_[:], a_[:], t_[:], add)
                        for bb, a_, ar_, t_, q_, u_, v_ in ext:
                            nc.gpsimd.tensor_tensor(q_[:], q_[:], v_[:], add)
                        for bb, a_, ar_, t_, q_, u_, v_ in ext:
                            smul(u_, 7, bb, act_mults >= 4)
                        for bb, a_, ar_, t_, q_, u_, v_ in ext:
                            smul(t_, 3, bb, False)
                        for bb, a_, ar_, t_, q_, u_, v_ in ext:
                            nc.vector.tensor_tensor(a_[:], a_[:], t_[:], add)
                        for bb, a_, ar_, t_, q_, u_, v_ in ext:
                            nc.gpsimd.tensor_tensor(q_[:], q_[:], u_[:], add)
                        for bb, a_, ar_, t_, q_, u_, v_ in ext:
                            if final_pool:
                                nc.gpsimd.tensor_tensor(
                                    ar_[:], a_[:], q_[:], add)
                            else:
                                nc.vector.tensor_tensor(
                                    ar_[:], a_[:], q_[:], add)
                    elif combine == "twoop":
                        for e in range(E):
                            for bb, a_, ar_, t_ in pr:
                                s = rw_sb[:, bb * E + e:bb * E + e + 1]
                                on_act = E - 1 - act_mults <= e < E - 1
                                if e == 0:
                                    if not act_e0:
                                        nc.vector.tensor_scalar(
                                            a_[:], wt_sb[0][:], s, None, mult)
                                elif e == E - 1 and bb < act_last:
                                    pass
                                elif e == 1 and bb < act_third:
                                    pass
                                elif on_act:
                                    nc.scalar.mul(t_[:], wt_sb[e][:], s)
                                else:
                                    nc.vector.tensor_scalar(
                                        t_[:], wt_sb[e][:], s, None, mult)
                            if e > 0:
                                for bb, a_, ar_, t_ in pr:
                                    dst = ar_ if e == E - 1 else a_
                                    src = (actp[bb] if act_e0 and e == 1
                                           else a_)
                                    rhs_t = t_
                                    if e == E - 1 and bb < act_last:
                                        rhs_t = actq[bb]
                                    elif e == 1 and bb < act_third:
                                        rhs_t = actr[bb]
                                    if e == E - 1 and pool_final:
                                        nc.gpsimd.tensor_tensor(
                                            dst[:], src[:], rhs_t[:], add)
                                    else:
                                        nc.vector.tensor_tensor(
                                            dst[:], src[:], rhs_t[:], add)
                    elif combine == "twoop2":
                        # pair-merged accumulators: adds run on [128, 2*WCOL]
                        # (both pair members in one instruction, halving DVE
                        # add instruction count); mults stay per-sample.
                        fdt = f16 if use_f16 else f32
                        ap_ = agg_pool.tile([128, GSZ * WCOL], fdt, tag="apair")
                        tp_ = agg_pool.tile([128, GSZ * WCOL], fdt, tag="tpair")
                        for gi in range(GSZ):
                            pair_tiles[b + gi] = (ap_, gi * WCOL)
                        for e in range(E):
                            for gi, (bb, a_, ar_, t_) in enumerate(pr):
                                s = rw_sb[:, bb * E + e:bb * E + e + 1]
                                dst = ap_ if e == 0 else tp_
                                on_act = E - 1 - act_mults <= e < E - 1
                                if on_act:
                                    nc.scalar.mul(
                                        dst[:, gi * WCOL:(gi + 1) * WCOL],
                                        wt_sb[e][:], s)
                                else:
                                    nc.vector.tensor_scalar(
                                        dst[:, gi * WCOL:(gi + 1) * WCOL],
                                        wt_sb[e][:], s, None, mult)
                            if e > 0:
                                nc.vector.tensor_tensor(
                                    ap_[:], ap_[:], tp_[:], add)
                    else:
                        for e in range(E):
                            for bb, a_, ar_, t_ in pr:
                                s = rw_sb[:, bb * E + e:bb * E + e + 1]
                                if e == 0:
                                    nc.vector.tensor_scalar(
                                        a_[:], wt_sb[0][:], s, None, mult)
                                elif e < E - 1:
                                    nc.vector.scalar_tensor_tensor(
                                        a_[:], wt_sb[e][:], s, a_[:], mult, add)
                                elif micro and b == BPC - GSZ and "dve0" not in probe:
                                    for k3 in range(KI):
                                        cs = slice(k3 * C_OUT, (k3 + 1) * C_OUT)
                                        nc.vector.scalar_tensor_tensor(
                                            ar_[:, cs], wt_sb[e][:, cs], s,
                                            a_[:, cs], mult, add)
                                else:
                                    nc.vector.scalar_tensor_tensor(
                                        ar_[:], wt_sb[e][:], s, a_[:], mult, add)
                aoff = 0
                if pair:
                    pt = pair_tiles[b % GSZ if "dve0" in probe else b]
                    if combine == "twoop2":
                        aggT_r, aoff = pt
                        aggT = aggT_r
                    else:
                        aggT, aggT_r = pt
                    sc = lambda e: rw_sb[:, b * E + e:b * E + e + 1]
                else:
                    aggT = agg_pool.tile([128, WCOL], f16 if use_f16 else f32)
                    aggT_r = agg_pool.tile([128, WCOL], dmm, tag="aggr")
                    sc = lambda e: rw_sb[:, b * E + e:b * E + e + 1]
                if not pair:
                    nc.vector.tensor_scalar(
                        aggT[:, 0:spl], wt_sb[0][:, 0:spl], sc(0), None, mult
                    )
                    for e in range(1, E - 1):
                        nc.vector.scalar_tensor_tensor(
                            aggT[:, 0:spl], wt_sb[e][:, 0:spl], sc(e),
                            aggT[:, 0:spl], mult, add,
                        )
                    nc.vector.scalar_tensor_tensor(
                        aggT_r[:, 0:spl], wt_sb[E - 1][:, 0:spl], sc(E - 1),
                        aggT[:, 0:spl], mult, add,
                    )
                if spl < WCOL:
                    gw = WCOL - spl
                    gtmp = agg_pool.tile([128, gw], f16 if use_f16 else f32,
                                         tag="gtmp")
                    scb = lambda e: sc(e).to_broadcast((128, gw))
                    nc.gpsimd.tensor_tensor(
                        aggT[:, spl:], wt_sb[0][:, spl:], scb(0), mult)
                    for e in range(1, E - 1):
                        nc.gpsimd.tensor_tensor(
                            gtmp[:], wt_sb[e][:, spl:], scb(e), mult)
                        nc.gpsimd.tensor_tensor(
                            aggT[:, spl:], aggT[:, spl:], gtmp[:], add)
                    nc.gpsimd.tensor_tensor(
                        gtmp[:], wt_sb[E - 1][:, spl:], scb(E - 1), mult)
                    nc.gpsimd.tensor_tensor(
                        aggT_r[:, spl:], aggT[:, spl:], gtmp[:], add)

                # ---- load x_b ----
                if "dmahalf" in probe and b % 2 == 1:
                    x_sb = x_prev
                else:
                    x_sb = x_pool.tile([128, KI * HW], dmm)
                    for ki in range(KI):
                        src = x_d[b, ki * 128:(ki + 1) * 128, :]
                        xi = _fence(nc.sync.dma_start(x_sb[:, ki * HW:(ki + 1) * HW],
                                                      src if use_f16 else src.bitcast(f32r)))
                        if micro and b < 2:
                            for wd in wt_dmas:
                                add_dep_helper(xi.ins, wd.ins,
                                               reason="x after wt (head trim)")
                    x_prev = x_sb

                # ---- per-sample GEMM ----
                ncol = 196 if "pecols" in probe else NCH
                for mo in range(MO):
                    if out_merge:
                        o_mo = out_pool.tile([128, HW], dio)
                    for n in range(NSPLIT):
                        ps = ps_pool.tile([128, NCH], f32)
                        nki = 2 if "pek2" in probe else KI
                        for ki in range(nki):
                            lhs = aggT_r[:, aoff + ki * C_OUT + mo * 128:
                                         aoff + ki * C_OUT + (mo + 1) * 128]
                            rhs = x_sb[:, ki * HW + n * NCH:
                                       ki * HW + n * NCH + ncol]
                            nc.tensor.matmul(
                                ps[:, 0:ncol], lhs, rhs,
                                start=(ki == 0), stop=(ki == nki - 1),
                            )
                        if out_merge:
                            dst = o_mo[:, n * NCH:n * NCH + ncol]
                            if mo * NSPLIT + n < evac_dve:
                                nc.vector.tensor_scalar(
                                    dst, ps[:, 0:ncol], 1.0, None, mult)
                            else:
                                nc.scalar.copy(dst, ps[:, 0:ncol])
                            if n == NSPLIT - 1:
                                cur_out_dmas.append(nc.sync.dma_start(
                                    out_d[b, mo * 128:(mo + 1) * 128, :],
                                    o_mo[:],
                                ))
                            continue
                        o_sb = out_pool.tile([128, NCH], dio)
                        nc.scalar.copy(o_sb[:, 0:ncol], ps[:, 0:ncol])
                        if "dmahalf" in probe and n != 0:
                            continue
                        if "pecols" in probe:
                            for half in range(2):
                                cur_out_dmas.append(nc.sync.dma_start(
                                    out_d[b,
                                          mo * 128:(mo + 1) * 128,
                                          n * NCH + half * 196:
                                          n * NCH + (half + 1) * 196],
                                    o_sb[:, 0:196],
                                ))
                        else:
                            cur_out_dmas.append(nc.sync.dma_start(
                                out_d[b, mo * 128:(mo + 1) * 128,
                                      n * NCH:(n + 1) * NCH],
                                o_sb[:],
                            ))
    nc.compile()
    return nc


def kernel(x: np.ndarray, routing_weights: np.ndarray, weight: np.ndarray,
           _trace: bool = False):
    from concourse.bass_utils import run_bass_kernel_spmd

    x = np.asarray(x, dtype=np.float32)
    routing_weights = np.ascontiguousarray(np.asarray(routing_weights, dtype=np.float32))
    weight = np.asarray(weight, dtype=np.float32)

    if "nc" not in _cache:
        _cache["nc"] = _build()
    nc = _cache["nc"]

    np_io = np.float16 if USE_F16 else np.float32

    # wt[e, p, ki*384 + o] = weight[e, o, ki*128 + p]
    wt = np.ascontiguousarray(
        weight.reshape(E, C_OUT, KI, 128).transpose(0, 3, 2, 1)
        .reshape(E, 128, WCOL).astype(np_io)
    )
    x_r = np.ascontiguousarray(x.reshape(B, C_IN, HW).astype(np_io))

    in_maps = []
    for c in range(N_CORES):
        sl = slice(c * BPC, (c + 1) * BPC)
        in_maps.append({
            "x": x_r[sl],
            "rw": np.ascontiguousarray(routing_weights[sl].reshape(1, BPC * E)),
            "wt": wt,
        })

    res = run_bass_kernel_spmd(nc, in_maps, core_ids=list(range(N_CORES)),
                               trace=_trace)
    out = np.concatenate([res.results[c]["out"] for c in range(N_CORES)], axis=0)
    if _trace:
        _cache["last_result"] = res
    return out.reshape(B, C_OUT, H, W).astype(np.float32)


if __name__ == "__main__":
    rng = np.random.default_rng(0)
    x = rng.standard_normal((B, C_IN, H, W), dtype=np.float32)
    rw = rng.random((B, E), dtype=np.float32)
    w = rng.standard_normal((E, C_OUT, C_IN), dtype=np.float32)
    got = kernel(x, rw, w)
    agg = np.einsum('be,eoi->boi', rw, w)
    want = np.einsum('boi,bihw->bohw', agg, x.reshape(B, C_IN, H, W))
    err = np.abs(got - want).max() / np.abs(want).max()
    print("rel err:", err)

